# revision 4
# baseline (speedup 1.0000x reference)
"""Causal self-attention (S=2048, D=2048, H=32 heads, Dh=64) on 8 TRN2 cores.

Strategy: tensor-parallel over heads (4 heads/core).
  - Host pre-transposes x -> xT [D, S] and casts inputs to bf16.
  - Per core: Wqkv columns for its 4 heads (Q,K cols used as lhsT producing
    Q^T/K^T directly; V cols used as rhs producing V in natural layout with a
    ones column appended for softmax denominators), Wout rows for its
    features.
  - Attention computed in "transposed world": logits^T tiles [S_k=128,
    S_q=512] = K_h^T.T @ Q_h^T, exp on ScalarE (scale=1/8 folded in), 0/1
    causal mask multiply on diagonal tiles, attn^T accumulated as
    [V|1].T @ probs^T giving [64+1, 512] (row 64 = softmax denominator).
  - Normalize via reciprocal + DRAM-roundtrip partition-broadcast, cast bf16.
  - Out projection: attn^T chunks as lhsT, Wout rows as rhs -> partial [S, D]
    per core, summed on host (+bias).

Self-contained: only concourse/numpy/ml_dtypes imports.
"""
from contextlib import ExitStack

import numpy as np
import orjson
import ml_dtypes

import concourse.bass as bass
import concourse.tile as tile
from concourse import mybir
from concourse.bass_utils import run_bass_kernel_spmd
from concourse.vector_clock import ScopedClock, VectorClock

S = 2048
D = 2048
DH = 64
NH = 32
NCORES = 8
HPC = NH // NCORES          # heads per core = 4
KC = D // 128               # contraction chunks = 16
BF16 = ml_dtypes.bfloat16
F32 = mybir.dt.float32
BF = mybir.dt.bfloat16


class _PatchedTileContext(tile.TileContext):
    """Workaround: walrus in this container allows only ONE sync wait per
    CTRL instruction; stock TileContext puts the whole global clock on the
    final drain.  Split the waits across single-wait SP nops instead."""

    def _drain_and_barrier(self, tick_clock, wait_clock):
        gc = tick_clock.global_clock
        n = len(gc)
        for proc in range(n):
            tick = gc[proc]
            if tick <= 0:
                continue
            vec = [0] * n
            vec[proc] = tick
            inst = self.nc.sync.nop(nofuse=True, hint=f"drain_wait_p{proc}")
            wait_clock.add_sem_waits(inst.ins, ScopedClock({None: VectorClock(vec)}))
        self.nc.sync.drain()
        self.nc.all_engine_barrier()
        assert self.sems is not None
        popped = self.nc._tile_sem_poison_stack.pop()
        assert popped is self._sem_poison
        self.nc.clear_and_free_semaphores(list(self.sems.allocated().values()))
        self.nc.all_engine_barrier()


def _legalize_sync_waits(nc, max_waits: int = 1):
    """Split any instruction with >max_waits sem waits into preceding
    same-engine single-wait NoOps (this walrus rejects multi-wait
    instructions of every class)."""
    j = orjson.loads(mybir.module_to_json_bytes(nc.m))
    counter = 0
    changed = False
    for func in j["functions"]:
        for blk in func["blocks"]:
            new_insts = []
            for inst in blk["instructions"]:
                si = inst.get("sync_info")
                waits = si.get("on_wait") if si else None
                if waits and len(waits) > max_waits:
                    changed = True
                    for w in waits[:-max_waits]:
                        counter += 1
                        new_insts.append({
                            "debug": inst.get("debug", 0),
                            "engine": inst["engine"],
                            "ins": [],
                            "name": f"LW-{counter}",
                            "opcode": "NoOp",
                            "outs": [],
                            "sync_info": {"on_update": [], "on_wait": [w]},
                            "text_hint": "legalize_wait",
                        })
                    si["on_wait"] = waits[-max_waits:]
                new_insts.append(inst)
            blk["instructions"] = new_insts
    if changed:
        nc.m = mybir.module_from_json_bytes(orjson.dumps(j))
    return nc


def build_nc():
    nc = bass.Bass()
    xT = nc.declare_dram_parameter("xT", [D, S], BF, isOutput=False)
    wqk = nc.declare_dram_parameter("wqk", [D, 2 * HPC * DH], BF, isOutput=False)
    wv = nc.declare_dram_parameter("wv", [D, HPC * DH], BF, isOutput=False)
    wout = nc.declare_dram_parameter("wout", [HPC * DH, D], BF, isOutput=False)
    masks = nc.declare_dram_parameter("masks", [4, 128, 512], BF, isOutput=False)
    out = nc.declare_dram_parameter("out", [S, D], F32, isOutput=True)

    with _PatchedTileContext(nc) as tc, ExitStack() as ctx:
        const = ctx.enter_context(tc.tile_pool(name="const", bufs=1))
        probs_pool = ctx.enter_context(tc.tile_pool(name="probs", bufs=3))
        rec_pool = ctx.enter_context(tc.tile_pool(name="rec", bufs=2))
        bc_pool = ctx.enter_context(tc.tile_pool(name="bc", bufs=2))
        osb_pool = ctx.enter_context(tc.tile_pool(name="osb", bufs=4))
        dr_pool = ctx.enter_context(tc.tile_pool(name="dr", bufs=2, space="DRAM"))

        xT_sb = const.tile([128, KC, S], BF)
        wqk_sb = const.tile([128, KC, 512], BF)
        wv_sb = const.tile([128, KC, 256], BF)
        wout_sb = const.tile([128, 2, S], BF)
        mask_sb = const.tile([128, 4, 512], BF)
        qt_sb = const.tile([128, 2, S], BF)
        kt_sb = const.tile([128, 2, S], BF)
        v_sb = const.tile([128, KC, HPC * 65], BF)
        attnT_sb = [
            const.tile([128, 2, 512], BF, tag=f"attnT{j}", name=f"attnT{j}")
            for j in range(4)
        ]

        # ---- constant DMAs (interleaved per contraction chunk) ----
        for k in range(KC):
            nc.sync.dma_start(wqk_sb[:, k, :], wqk[k * 128:(k + 1) * 128, :])
            nc.sync.dma_start(xT_sb[:, k, :], xT[k * 128:(k + 1) * 128, :])
        for k in range(KC):
            nc.sync.dma_start(wv_sb[:, k, :], wv[k * 128:(k + 1) * 128, :])
        # ones columns for the softmax denominators
        nc.vector.memset(
            v_sb[:].rearrange("p k (h x) -> p k h x", h=HPC)[:, :, :, 64:65], 1.0
        )
        for m in range(4):
            nc.sync.dma_start(mask_sb[:, m, :], masks[m])
        for c in range(2):
            nc.sync.dma_start(wout_sb[:, c, :], wout[c * 128:(c + 1) * 128, :])

        # ---- phase 1: projections ----
        with (
            tc.tile_pool(name="qkps", bufs=4, space="PSUM") as qkps,
            tc.tile_pool(name="vps", bufs=2, space="PSUM") as vps,
        ):
            # qk^T = Wqk.T @ x.T : psum [128 rows of qk^T, 512 cols of S]
            for m in range(4):
                pss = [
                    qkps.tile([128, 512], F32, name=f"qk_ps_{m}_{s}", tag="qk_ps")
                    for s in range(4)
                ]
                for k in range(KC):
                    for s in range(4):
                        nc.tensor.matmul(
                            pss[s],
                            wqk_sb[:, k, m * 128:(m + 1) * 128],
                            xT_sb[:, k, s * 512:(s + 1) * 512],
                            start=(k == 0), stop=(k == KC - 1),
                            skip_group_check=True,
                        )
                dest = qt_sb if m < 2 else kt_sb
                for s in range(4):
                    nc.scalar.copy(dest[:, m % 2, s * 512:(s + 1) * 512], pss[s])
            # V = x @ Wv : psum [128 rows of S, 256]
            for sv in range(KC):
                psv = vps.tile([128, 256], F32)
                for k in range(KC):
                    nc.tensor.matmul(
                        psv,
                        xT_sb[:, k, sv * 128:(sv + 1) * 128],
                        wv_sb[:, k, :],
                        start=(k == 0), stop=(k == KC - 1),
                        skip_group_check=True,
                    )
                nc.vector.tensor_copy(
                    v_sb[:, sv, :].rearrange("p (h x) -> p h x", h=HPC)[:, :, 0:64],
                    psv[:].rearrange("p (h x) -> p h x", h=HPC),
                )

        # ---- phases 2+3: attention + output projection, pipelined per j ----
        with (
            tc.tile_pool(name="lg", bufs=2, space="PSUM") as lg,
            tc.tile_pool(name="at", bufs=2, space="PSUM") as at,
            tc.tile_pool(name="ops", bufs=2, space="PSUM") as ops,
        ):
            for j in range(4):
                n_i = 4 * (j + 1)
                for h in range(HPC):
                    hp = (h % 2) * 64
                    hc = h // 2
                    attn_ps = at.tile([65, 512], F32)
                    for g in range(n_i // 2):
                        lg_ps = lg.tile([128, 1024], F32)
                        for d in range(2):
                            i = 2 * g + d
                            nc.tensor.matmul(
                                lg_ps[:, d * 512:(d + 1) * 512],
                                kt_sb[hp:hp + 64, hc, i * 128:(i + 1) * 128],
                                qt_sb[hp:hp + 64, hc, j * 512:(j + 1) * 512],
                                start=True, stop=True,
                                skip_group_check=True,
                            )
                        probs = probs_pool.tile([128, 1024], BF, tag="probs")
                        nc.scalar.activation(
                            probs[:], lg_ps[:],
                            mybir.ActivationFunctionType.Exp, scale=0.125,
                        )
                        for d in range(2):
                            i = 2 * g + d
                            mrel = i - 4 * j
                            if mrel >= 0:  # diagonal tile: causal mask
                                nc.vector.tensor_mul(
                                    probs[:, d * 512:(d + 1) * 512],
                                    probs[:, d * 512:(d + 1) * 512],
                                    mask_sb[:, mrel, :],
                                )
                            nc.tensor.matmul(
                                attn_ps[:],
                                v_sb[:, i, h * 65:(h + 1) * 65],
                                probs[:, d * 512:(d + 1) * 512],
                                start=(i == 0), stop=(i == n_i - 1),
                                skip_group_check=True,
                            )
                    rec = rec_pool.tile([1, 512], F32, tag="rec")
                    nc.vector.reciprocal(rec[:], attn_ps[64:65, :])
                    rec_dr = dr_pool.tile([1, 512], F32)
                    nc.sync.dma_start(rec_dr[:], rec[:])
                    bc = bc_pool.tile([64, 512], F32, tag="bc")
                    nc.sync.dma_start(bc[:], rec_dr[0:1, :].partition_broadcast(64))
                    nc.vector.tensor_mul(
                        attnT_sb[j][hp:hp + 64, hc, :], attn_ps[0:64, :], bc[:]
                    )
                # out rows for this j block: out[mo*128:(mo+1)*128, :]
                for mo in range(4 * j, 4 * j + 4):
                    for n in range(4):
                        pso = ops.tile([128, 512], F32)
                        for c in range(2):
                            nc.tensor.matmul(
                                pso[:],
                                attnT_sb[j][:, c, (mo % 4) * 128:(mo % 4 + 1) * 128],
                                wout_sb[:, c, n * 512:(n + 1) * 512],
                                start=(c == 0), stop=(c == 1),
                                skip_group_check=True,
                            )
                        ob = osb_pool.tile([128, 512], F32, tag="ob")
                        if n % 2 == 0:
                            nc.vector.tensor_copy(ob[:], pso[:])
                        else:
                            nc.scalar.copy(ob[:], pso[:])
                        nc.sync.dma_start(
                            out[mo * 128:(mo + 1) * 128, n * 512:(n + 1) * 512], ob[:]
                        )

    _legalize_sync_waits(nc)
    return nc


_NC_CACHE = None


def _get_nc():
    global _NC_CACHE
    if _NC_CACHE is None:
        _NC_CACHE = build_nc()
    return _NC_CACHE


def make_inputs(x, Wqkv, Wout, bias):
    xT = np.ascontiguousarray(x.T).astype(BF16)
    r = np.arange(128)[:, None]
    c = np.arange(512)[None, :]
    masks = np.stack(
        [(c >= r + m * 128).astype(np.float32) for m in range(4)]
    ).astype(BF16)
    in_maps = []
    for core in range(NCORES):
        h0 = core * HPC * DH          # 256 cols per core per q/k/v section
        wq = Wqkv[:, h0:h0 + HPC * DH]
        wk = Wqkv[:, D + h0:D + h0 + HPC * DH]
        wv_ = Wqkv[:, 2 * D + h0:2 * D + h0 + HPC * DH]
        in_maps.append({
            "xT": xT,
            "wqk": np.concatenate([wq, wk], axis=1).astype(BF16),
            "wv": wv_.astype(BF16),
            "wout": Wout[h0:h0 + HPC * DH, :].astype(BF16),
            "masks": masks,
        })
    return in_maps


def kernel(x, Wqkv, Wout, bias, _trace=False, _trace_kwargs=None):
    x = np.asarray(x)
    Wqkv = np.asarray(Wqkv)
    Wout = np.asarray(Wout)
    bias = np.asarray(bias)
    nc = _get_nc()
    in_maps = make_inputs(x, Wqkv, Wout, bias)
    res = run_bass_kernel_spmd(
        nc, in_maps, core_ids=list(range(NCORES)),
        trace=_trace, **(_trace_kwargs or {}),
    )
    acc = np.zeros((S, D), np.float64)
    for core in range(NCORES):
        acc += res.results[core]["out"].astype(np.float64)
    out = (acc + bias.astype(np.float64)[None, :]).astype(np.float32)
    if _trace:
        kernel._last_result = res
    return out
